# revision 34
# baseline (speedup 1.0000x reference)
"""Trainium2 Bass kernel for nn_Loss_34608846471397 (center-loss style).

Strategy: data-parallel over batch, 8 cores x 4096 rows.  Host bin-packs
each core's classes into 8 blocks of exactly 512 rows (<=128 classes
each, no pad rows), so the per-row center gather becomes a one-hot
matmul on the PE: G = OT.T @ C_blk per 128-row tile.  A DoubleRow fp8
matmul fuses the subtract: PSUM = OT.T @ C_blk + (-Id).T @ F = G - F in
one instruction (~216ns/tile warm).  dist^2 = row-reduce(diff^2) split
across Act (Square+accum) and DVE (bn_stats; d2 = M2_e + M2_o +
256*(mu_e^2 + mu_o^2) recombined at the end).  A dummy Sqrt up front
makes the act-table pass load the one table containing Square+Sqrt+Copy
during the initial DMA wait; PE heater matmuls ramp the tensor engine's
p-state over the same window.  Rows with labels C-2/C-1 are routed to
core 7's last tile; one tiny mask matmul there produces the inter-loss
class sums.  Inputs ship as one flat fp8 blob (one-hots, -Id, fp8
centers, fp8 features, bf16 mask/last-tile rows via bitcast) in 5
graduated DMA chunks so no tile ever waits on data.
"""

import os
import sys

for _p in ("/opt/trn_rl_repo", "/root/.axon_site/_ro/trn_rl_repo"):
    if os.path.isdir(_p) and _p not in sys.path:
        sys.path.insert(0, _p)

import numpy as np
import ml_dtypes

import concourse.bacc as bacc
import concourse.bass as bass
import concourse.tile as tile
from concourse import mybir

B = 32768
D = 512
C = 1000
N_CORES = 8
P = 128
NBLK = 8          # class blocks per core (exact-fill bin-packed)
TPB = 4           # tiles per block (512 row slots)
NT = NBLK * TPB   # 36 tiles per core
ROWS = B // N_CORES
CAP = TPB * P     # 512 rows per block
# fused bytes per block: ot (TPB+1 slots of 128) + cf (1+TPB slots of 512)
OTB = (TPB + 1) * P
CFB = (TPB + 1) * D
BPB = OTB + CFB

F8 = ml_dtypes.float8_e4m3
BF16 = ml_dtypes.bfloat16

# per-tile compute mode: dr_act = Act Square+accum; dr_bn = DVE bn_stats.
# Tail is all-bn so Act finishes early (Sqrt table load overlaps).
MODES = ["dr_act" if t % 2 == 0 else "dr_bn" for t in range(28)] \
    + ["dr_act"] + ["dr_bn"] * 3
NB = sum(1 for m in MODES if m == "dr_bn")
NA = NT - NB

_cache = {}


def _build(ncores=N_CORES):
    nc = bacc.Bacc("TRN2", target_bir_lowering=False, debug=False,
                   num_devices=ncores)
    f32 = mybir.dt.float32
    f8 = mybir.dt.float8e4
    bf = mybir.dt.bfloat16
    AF = mybir.ActivationFunctionType
    OP = mybir.AluOpType
    AX = mybir.AxisListType

    # flat blob: [block0 | mflast-as-bytes | block1..7]
    MFB = (2 + D) * 2
    TOT = NBLK * BPB + MFB
    fused = nc.dram_tensor("fused", [P, TOT], f8, kind="ExternalInput")

    intra_out = nc.dram_tensor("intra_out", [P, 1], f32, kind="ExternalOutput")
    sums_out = nc.dram_tensor("sums_out", [2, D], f32, kind="ExternalOutput")

    with tile.TileContext(nc) as tc:
        with (
            tc.tile_pool(name="fused", bufs=1) as fpool,
            tc.tile_pool(name="small", bufs=1) as mpool,
            tc.tile_pool(name="sq", bufs=4) as qpool,
            tc.tile_pool(name="psum", bufs=7, space="PSUM") as ppool,
            tc.tile_pool(name="psums", bufs=1, space="PSUM") as spool,
        ):
            # graduated byte-range chunks so no tile ever stalls; mflast is
            # embedded in chunk 0 (viewed through a bf16 bitcast).
            ranges = [(0, BPB + MFB), (BPB + MFB, 2 * BPB + MFB),
                      (2 * BPB + MFB, 3 * BPB + MFB),
                      (3 * BPB + MFB, 5 * BPB + MFB),
                      (5 * BPB + MFB, 8 * BPB + MFB)]
            f_sb = []
            for (lo, hi) in ranges:
                ft = fpool.tile([P, hi - lo], f8, tag=f"f{lo}")
                nc.sync.dma_start(out=ft[:], in_=fused[:, lo:hi])
                f_sb.append((lo, hi, ft))

            c0 = f_sb[0][2]
            mask_v = c0[:, BPB:BPB + 4].bitcast(bf)
            flast_v = c0[:, BPB + 4:BPB + MFB].bitcast(bf)

            def views(b):
                boff = b * BPB + (MFB if b >= 1 else 0)
                for lo, hi, ft in f_sb:
                    if lo <= boff < hi:
                        base = ft[:, boff - lo:boff - lo + BPB]
                        break
                ot = base[:, 0:OTB].rearrange("p (s d) -> p s d", s=TPB + 1)
                cf = base[:, OTB:].rearrange("p (s d) -> p s d", s=TPB + 1)
                return ot, cf

            # Dummy Sqrt first: forces the act-table pass to load a
            # sqrt-capable table (which also contains square and copy), so
            # the whole kernel needs exactly one table load, done during the
            # initial DMA wait instead of on the critical tail.
            warm = mpool.tile([P, 1], f32, tag="warm")
            nc.vector.memset(warm[:], 1.0)
            warm2 = mpool.tile([P, 1], f32, tag="warm2")
            nc.scalar.activation(out=warm2[:], in_=warm[:], func=AF.Sqrt)

            # PE heaters: keep the tensor engine busy during the initial DMA
            # wait so it reaches its fast p-state before the first real tile.
            # They write the sums PSUM tile, which the real sums matmul
            # overwrites (start=True) afterwards.
            hb = mpool.tile([P, D], bf, tag="heat")
            nc.vector.memset(hb[:], 0.5)
            spsum = spool.tile([2, D], f32)
            for _ in range(4):
                nc.tensor.matmul(out=spsum[:], lhsT=hb[:, 0:2], rhs=hb[:],
                                 start=True, stop=True)

            # single dist2 tile: act columns 0..NA-1 via accum, bn columns
            # NA.. via the two-part fixup below
            NB1 = NB - 3
            dist2 = mpool.tile([P, NT], f32, tag="d2")
            stats1 = mpool.tile([P, NB1, 6], f32, tag="bn1")
            stats2 = mpool.tile([P, 3, 6], f32, tag="bn2")

            # inter-loss class sums (nonzero only on core 7); emitted early
            # so the Act copy and output DMA run during the main stream.
            nc.tensor.matmul(out=spsum[:], lhsT=mask_v, rhs=flast_v,
                             start=True, stop=True)
            sums_sb = mpool.tile([2, D], f32, tag="sums")
            nc.scalar.copy(out=sums_sb[:], in_=spsum[:])
            nc.sync.dma_start(out=sums_out[:], in_=sums_sb[:])

            na = 0
            nb = 0
            for t in range(NT):
                b, i = divmod(t, TPB)
                ot, cf = views(b)
                ps = ppool.tile([P, D], f32)
                # PSUM = OT.T @ C_b + (-Id).T @ F  (= G - F)
                nc.tensor.matmul(
                    out=ps[:],
                    lhsT=ot[:, i:TPB + 1:(TPB - i), :],
                    rhs=cf[:, 0:(2 + i):(1 + i), :],
                    start=True, stop=True,
                    perf_mode=mybir.MatmulPerfMode.DoubleRow,
                )
                if MODES[t] == "dr_act":
                    sq = qpool.tile([P, D], bf, tag="sq")
                    nc.scalar.activation(out=sq[:], in_=ps[:],
                                         func=AF.Square,
                                         accum_out=dist2[:, na:na + 1])
                    na += 1
                else:
                    if nb < NB1:
                        nc.vector.bn_stats(out=stats1[:, nb, :], in_=ps[:])
                    else:
                        nc.vector.bn_stats(out=stats2[:, nb - NB1, :],
                                           in_=ps[:])
                    nb += 1
            assert na == NA and nb == NB

            # bn fixup: d2 = M2_even + M2_odd + 256*(mean_even^2 + mean_odd^2)
            # done in two parts so the bulk overlaps the tail bn tiles.
            def fixup(st, n, dcol):
                me = st[:, :, 1]
                ve = st[:, :, 2]
                mo = st[:, :, 4]
                vo = st[:, :, 5]
                e2 = mpool.tile([P, n], f32, tag=f"e2{n}")
                o2 = mpool.tile([P, n], f32, tag=f"o2{n}")
                ss = mpool.tile([P, n], f32, tag=f"ss{n}")
                vv = mpool.tile([P, n], f32, tag=f"vv{n}")
                nc.vector.tensor_tensor(out=e2[:], in0=me, in1=me, op=OP.mult)
                nc.vector.tensor_tensor(out=o2[:], in0=mo, in1=mo, op=OP.mult)
                nc.vector.tensor_tensor(out=ss[:], in0=e2[:], in1=o2[:],
                                        op=OP.add)
                nc.vector.tensor_tensor(out=vv[:], in0=ve, in1=vo, op=OP.add)
                nc.vector.scalar_tensor_tensor(
                    out=dist2[:, dcol:dcol + n], in0=ss[:],
                    scalar=float(D // 2), in1=vv[:], op0=OP.mult, op1=OP.add)

            fixup(stats1, NB1, NA)
            fixup(stats2, 3, NA + NB1)

            # epilogue: one Sqrt+accum over all columns gives the intra
            # partial directly (no clip needed: exact packing leaves no
            # zero pad rows, so distances are O(10)).
            dist = mpool.tile([P, NT], f32, tag="dist")
            intra_col = mpool.tile([P, 1], f32, tag="intra")
            nc.scalar.activation(out=dist[:], in_=dist2[:], func=AF.Sqrt,
                                 accum_out=intra_col[:])
            nc.sync.dma_start(out=intra_out[:], in_=intra_col[:])

    nc.compile()
    return nc


def _pack_blocks(cnt, special):
    """Exact-fill bin-pack: NBLK blocks of exactly CAP rows, <= P classes.
    Classes with zero rows are dropped.  `special` classes (998/999 on core
    7) are forced to the END of the last block's class list."""
    bins = [[] for _ in range(NBLK)]
    rows = [0] * NBLK
    last = NBLK - 1
    rows[last] += sum(int(cnt[c]) for c in special)
    order = np.argsort(-cnt, kind="stable")
    for c in order:
        c = int(c)
        if cnt[c] == 0 or c in special:
            continue
        best = None
        for j in range(NBLK):
            limit = P - len(special) if j == last else P
            if rows[j] + cnt[c] <= CAP and len(bins[j]) < limit:
                if best is None or rows[j] < rows[best]:
                    best = j
        assert best is not None, "bin packing failed"
        bins[best].append(c)
        rows[best] += int(cnt[c])
    # repair to exactly CAP rows per bin by shuffling small classes
    for _ in range(10000):
        under = [j for j in range(NBLK) if rows[j] < CAP]
        if not under:
            break
        under.sort(key=lambda j: rows[j])
        a = under[0]
        moved = False
        for b in reversed(under[1:]):
            deficit = CAP - rows[b]
            limit = P - len(special) if b == last else P
            for c in sorted(bins[a], key=lambda c: cnt[c]):
                if cnt[c] <= deficit and len(bins[b]) < limit:
                    bins[a].remove(c)
                    bins[b].append(c)
                    rows[a] -= int(cnt[c])
                    rows[b] += int(cnt[c])
                    moved = True
                    break
            if moved:
                break
        assert moved, "bin repair failed"
    assert all(r == CAP for r in rows) and all(len(b) <= P for b in bins)
    for j in range(NBLK):
        if not bins[j]:
            bins[j].append(0 if 0 not in special else 1)
    bins[last].extend(special)
    assert len(bins[last]) > len(special)  # need a non-special pad class
    return bins


def _prep(features, labels, center):
    feats = np.ascontiguousarray(features, dtype=np.float32)
    labs = np.ascontiguousarray(labels, dtype=np.int64)
    cent = np.ascontiguousarray(center, dtype=np.float32)

    c8 = cent.astype(F8)                      # [C, D] fp8
    c8f = c8.astype(np.float32)
    c8_64 = c8.astype(np.float64)

    # route C-2/C-1 rows to core 7, split the rest contiguously
    sp_idx = np.where(labs >= C - 2)[0]
    rest = np.where(labs < C - 2)[0]
    assert len(sp_idx) <= P, len(sp_idx)
    cores = [rest[k * ROWS:(k + 1) * ROWS] for k in range(N_CORES - 1)]
    cores.append(np.concatenate([rest[(N_CORES - 1) * ROWS:], sp_idx]))
    assert all(len(ck) == ROWS for ck in cores)

    # per-class row indices (global)
    by_class = [[] for _ in range(C)]

    karange = np.arange(P)[:, None]
    negid = (-np.eye(P, dtype=np.float32)).astype(F8)

    in_maps = []
    corr = 0.0
    for k in range(N_CORES):
        idx = cores[k]
        lab_k = labs[idx]
        cnt = np.bincount(lab_k, minlength=C)
        special = [C - 2, C - 1] if k == N_CORES - 1 else []
        bins = _pack_blocks(cnt, special)

        cls_rows = {}
        order_in_core = np.argsort(lab_k, kind="stable")
        sorted_rows = idx[order_in_core]
        sorted_labs = lab_k[order_in_core]
        starts = np.searchsorted(sorted_labs, np.arange(C))
        ends = np.searchsorted(sorted_labs, np.arange(C), side="right")

        slots = np.empty(NT * P, dtype=np.int64)
        slot_lab = np.empty(NT * P, dtype=np.int64)   # class id per slot
        slot_k = np.empty(NT * P, dtype=np.int64)     # class slot-index
        pos = 0
        for b in range(NBLK):
            cls_list = bins[b]
            n_special = len(special) if b == NBLK - 1 else 0
            regular = cls_list if n_special == 0 else cls_list[:-n_special]
            ids = []
            labsl = []
            kidx = []
            for ci, c in enumerate(cls_list[:len(regular)]):
                rws = sorted_rows[starts[c]:ends[c]]
                ids.extend(rws)
                labsl.extend([c] * len(rws))
                kidx.extend([ci] * len(rws))
            npad = CAP - len(ids) - (0 if n_special == 0 else
                                     sum(int(cnt[c]) for c in special))
            assert npad >= 0, (k, b, npad)
            padlab = regular[-1]
            padk = len(regular) - 1
            ids.extend([-1] * npad)
            labsl.extend([padlab] * npad)
            kidx.extend([padk] * npad)
            corr += npad * float(
                np.clip(np.sqrt(np.sum(c8_64[padlab] ** 2)), 1e-12, 1e12))
            if n_special:
                for ci, c in enumerate(cls_list[-n_special:]):
                    rws = sorted_rows[starts[c]:ends[c]]
                    ids.extend(rws)
                    labsl.extend([c] * len(rws))
                    kidx.extend([len(regular) + ci] * len(rws))
            assert len(ids) == CAP
            slots[pos:pos + CAP] = ids
            slot_lab[pos:pos + CAP] = labsl
            slot_k[pos:pos + CAP] = kidx
            pos += CAP

        # padded feature rows, fp8
        fpad = np.zeros((NT * P, D), dtype=np.float32)
        real = slots >= 0
        fpad[real] = feats[slots[real]]
        f8pad = fpad.astype(F8)
        ftiles = f8pad.reshape(NT, P, D).transpose(1, 0, 2)  # [P, NT, D]

        k2 = slot_k.reshape(NT, P)                # [t, p] class slot index
        MFB = (2 + D) * 2
        TOT = NBLK * BPB + MFB
        fusedv = np.zeros((P, TOT), dtype=F8)
        for b in range(NBLK):
            boff = b * BPB + (MFB if b >= 1 else 0)
            for i in range(TPB):
                t = TPB * b + i
                ohot = (k2[t][None, :] == karange)
                fusedv[:, boff + i * P:boff + (i + 1) * P] = \
                    ohot.astype(np.float32).astype(F8)
            fusedv[:, boff + TPB * P:boff + (TPB + 1) * P] = negid
            cblk = np.zeros((P, D), dtype=np.float32)
            cls_list = bins[b]
            cblk[:len(cls_list)] = c8f[cls_list]
            fusedv[:, boff + OTB:boff + OTB + D] = cblk.astype(F8)
            for i in range(TPB):
                off = boff + OTB + (1 + i) * D
                fusedv[:, off:off + D] = ftiles[:, TPB * b + i, :]

        lab_last = slot_lab[(NT - 1) * P:]
        mfl = np.zeros((P, 2 + D), dtype=BF16)
        mfl[:, 0] = (lab_last == C - 2).astype(np.float32).astype(BF16)
        mfl[:, 1] = (lab_last == C - 1).astype(np.float32).astype(BF16)
        mfl[:, 2:] = fpad[(NT - 1) * P:].astype(BF16)
        fusedv[:, BPB:BPB + MFB] = mfl.view(np.uint8).view(F8)

        in_maps.append({"fused": fusedv})

    aux = {"corr": corr,
           "counts": np.bincount(labs, minlength=C).astype(np.float64)}
    return in_maps, aux


def _combine(results, aux, center):
    cent = np.asarray(center, dtype=np.float32)
    intra_sum = 0.0
    sums = np.zeros((2, D), dtype=np.float64)
    for r in results:
        intra_sum += float(r["intra_out"].sum(dtype=np.float64))
        sums += r["sums_out"].astype(np.float64)
    intra_loss = np.float32((intra_sum - aux["corr"]) / B)

    cen = np.empty((2, D), dtype=np.float32)
    for i, c in enumerate((C - 2, C - 1)):
        cnt = np.float32(max(aux["counts"][c], 1.0))
        cen[i] = (cent[c] + sums[i].astype(np.float32)) / cnt
    dvec = cen[0] - cen[1]
    d_last = np.float32(np.sqrt(np.sum(dvec * dvec, dtype=np.float32)))
    inter_loss = np.float32((2.0 / d_last) * (1.0 / (C * (C - 1))))
    return intra_loss, inter_loss


def kernel(features, labels, center, _trace=False):
    from concourse.bass_utils import run_bass_kernel_spmd
    if "nc" not in _cache:
        _cache["nc"] = _build()
    nc = _cache["nc"]
    in_maps, aux = _prep(features, labels, center)
    res = run_bass_kernel_spmd(nc, in_maps, core_ids=list(range(N_CORES)),
                               trace=_trace)
    if _trace:
        _cache["exec_time_ns"] = res.exec_time_ns
    return _combine(res.results, aux, center)
